# revision 1
# baseline (speedup 1.0000x reference)
"""Trainium2 Bass kernel for nn_Encoder_47167330845225.

Three embedding+LSTM encoders (source-comment, commit-msg, issue) + scalar
merge + final projection.  Data-parallel over the PR batch (B=64) across 8
NeuronCores; embedding tables and weights replicated.

Per-core layout choices:
  - everything bf16 except PSUM accumulation and final outputs (measured
    end-to-end relative error vs fp32 reference ~0.5%).
  - LSTM state kept "gate-transposed": hT/cT live as [128, 4, Nb] SBUF tiles
    (H on partitions) so each recurrence step needs no transposes at all.
  - recurrence matmul: weights stationary (lhsT = WhhT tile [128,128]),
    batch on the moving free dim.  z for one gate = one PSUM bank
    [128, 4*Nb] (4 m-tiles packed), gates ACT-processed one bank at a time.
  - x-projection X = emb @ WihT + b precomputed into DRAM (time-major,
    [T, 128, 16, Nb]) and injected into PSUM via identity-matmul.
  - tanh computed as 2*sigmoid(2x)-1 so ACT never swaps LUT tables.
  - gate order permuted host-side to [i, f, o, g] so each PSUM bank is a
    single ACT function.
"""

import math
import os

import numpy as np
import ml_dtypes

BF16 = ml_dtypes.bfloat16
P = 128
V, H, E = 32000, 512, 256
G = 4 * H                      # 2048 gate rows
B, NCOM, LSC, LCM, LIS = 64, 10, 128, 64, 32
NCORES = 8
BPC = B // NCORES              # 8 PRs per core
NSEQ = BPC * NCOM              # 80 commit sequences per core
MT = G // P                    # 16 m-tiles
KH = H // P                    # 4 k-tiles over H
KE = E // P                    # 2 k-tiles over E

# (name, T, Nb, chunk_steps)
_T_OVR = os.environ.get("BASSK_TSTEPS")
if _T_OVR:
    _TS = [int(x) for x in _T_OVR.split(",")]
else:
    _TS = [LSC, LCM, LIS]
CHAINS = [
    ("sc", _TS[0], NSEQ, 8),
    ("cm", _TS[1], NSEQ, 8),
    ("is", _TS[2], BPC, 32),
]
_DEBUG = int(os.environ.get("BASSK_DEBUG", "0"))
# phase gating for perf bisection: 3=full, 2=gather+proj, 1=gather only
_PHASE = int(os.environ.get("BASSK_PHASE", "3"))

# permute pytorch gate order i,f,g,o -> i,f,o,g so each 512-row block of z is
# one ACT function (sigmoid, sigmoid, sigmoid, tanh-as-sigmoid)
_GPERM = np.r_[0:H, H:2 * H, 3 * H:4 * H, 2 * H:3 * H]

_CACHE = {}


def _emit(tc, dram, xt, scratch):
    import concourse.bass as bass
    import concourse.mybir as mybir
    from concourse.masks import make_identity
    from contextlib import ExitStack

    dt = mybir.dt
    A = mybir.ActivationFunctionType
    OP = mybir.AluOpType
    nc = tc.nc

    with ExitStack() as ctx:
        const = ctx.enter_context(tc.tile_pool(name="const", bufs=1))

        # ---- persistent SBUF: weights, biases, indices, states ----
        w_sb, u_sb, b_sb, idx_sb, h_sb, c_sb = {}, {}, {}, {}, {}, {}
        for name, T, Nb, S in CHAINS:
            w = const.tile([P, KH, G], dt.bfloat16, tag=f"whh_{name}")
            nc.sync.dma_start(w[:], dram[f"whh_{name}"].rearrange("(k p) g -> p k g", p=P))
            w_sb[name] = w
            u = const.tile([P, KE, G], dt.bfloat16, tag=f"wih_{name}")
            nc.sync.dma_start(u[:], dram[f"wih_{name}"].rearrange("(k p) g -> p k g", p=P))
            u_sb[name] = u
            bb = const.tile([P, MT], dt.float32, tag=f"bias_{name}")
            nc.sync.dma_start(bb[:], dram[f"bias_{name}"].rearrange("(m p) -> p m", p=P))
            b_sb[name] = bb
            ntok = T * Nb
            ix = const.tile([P, ntok // 16], dt.int16, tag=f"idx_{name}")
            nc.sync.dma_start(ix[:], dram[f"idx_{name}"])
            idx_sb[name] = ix
            h = const.tile([P, KH, Nb], dt.bfloat16, tag=f"h_{name}")
            nc.vector.memset(h[:], 0.0)
            h_sb[name] = h
            c = const.tile([P, KH, Nb], dt.bfloat16, tag=f"c_{name}")
            nc.vector.memset(c[:], 0.0)
            c_sb[name] = c

        ident = const.tile([P, P], dt.bfloat16, tag="ident")
        make_identity(nc, ident[:])

        wm_sb = const.tile([P, KH, 4], dt.bfloat16, tag="wm")
        nc.sync.dma_start(wm_sb[:], dram["wm"].rearrange("(k p) c -> p k c", p=P))
        bm_sb = const.tile([1, 2], dt.float32, tag="bm")
        nc.sync.dma_start(bm_sb[:], dram["bm"])
        wfm_sb = const.tile([P, 2, H], dt.bfloat16, tag="wfm")
        nc.sync.dma_start(wfm_sb[:], dram["wf_m"].rearrange("c p m -> p c m"))
        wfh_sb = const.tile([P, 2, KH, H], dt.bfloat16, tag="wfh")
        nc.sync.dma_start(wfh_sb[:], dram["wf_h"].rearrange("c (k p) m -> p c k m", p=P))
        bf_sb = const.tile([P, KH, 2], dt.float32, tag="bf")
        nc.sync.dma_start(bf_sb[:], dram["bf"].rearrange("(m p) c -> p m c", p=P))

        # ---- phase A: gather, transpose, x-projection -> DRAM ----
        with tc.tile_pool(name="embt", bufs=1) as embt_pool, \
             tc.tile_pool(name="gat", bufs=6) as gat, \
             tc.tile_pool(name="chunk", bufs=2) as chunkp, \
             tc.tile_pool(name="ppsum", bufs=4, space="PSUM") as ppsum:
            for name, T, Nb, S in CHAINS:
                ntok = T * Nb
                GCH = S * Nb                      # tokens per gather/proj chunk
                assert GCH % 128 == 0 and T % S == 0
                nchunks = T // S
                embt = embt_pool.tile([P, nchunks, KE, GCH], dt.bfloat16,
                                      tag=f"embt_{name}")
                # hardware gather+transpose: embt[p, g, k, i] = tab[idx, k*128+p]
                for g in range(nchunks):
                    nc.gpsimd.dma_gather(
                        out_ap=embt[:, g, :, :],
                        in_ap=dram[f"tab_{name}"][:, :],
                        idxs_ap=idx_sb[name][:, g * (GCH // 16):(g + 1) * (GCH // 16)],
                        num_idxs=GCH,
                        num_idxs_reg=GCH,
                        elem_size=E,
                        transpose=True,
                        queue_num=0,
                    )
                if _PHASE < 2:
                    nc.sync.dma_start(
                        xt[name][0, :, :2, :min(Nb, GCH // MT)],
                        embt[:, 0, :, :min(Nb, GCH // MT)])
                    continue
                # projection: per chunk, sub-chunks of <=512 moving columns
                NSUB = GCH // 2 if GCH > 512 else GCH
                ssub = NSUB // Nb                 # steps per sub-chunk
                for ci in range(nchunks):
                    csb = chunkp.tile([P, S, MT, Nb], dt.bfloat16,
                                      tag=f"chunk_{'c' if Nb == NSEQ else name}")
                    for m in range(MT):
                        for hsub in range(GCH // NSUB):
                            zp = ppsum.tile([P, NSUB], dt.float32, tag="proj")
                            for k in range(KE):
                                nc.tensor.matmul(
                                    zp[:],
                                    lhsT=u_sb[name][:, k, m * P:(m + 1) * P],
                                    rhs=embt[:, ci, k, hsub * NSUB:(hsub + 1) * NSUB],
                                    start=(k == 0), stop=(k == KE - 1))
                            out_ap = csb[:, hsub * ssub:(hsub + 1) * ssub, m, :]
                            in_ap = zp[:].rearrange("p (s j) -> p s j", j=Nb)
                            if m % 2 == 0:
                                nc.vector.tensor_scalar(
                                    out_ap, in_ap, b_sb[name][:, m:m + 1], None, OP.add)
                            else:
                                nc.scalar.activation(
                                    out_ap, in_ap, A.Identity, bias=b_sb[name][:, m:m + 1])
                    nc.sync.dma_start(
                        xt[name][ci * S:(ci + 1) * S].rearrange("s p m j -> p s m j"),
                        csb[:])

        # ---- phase B: recurrences, interleaved ----
        with tc.tile_pool(name="zps", bufs=1, space="PSUM") as zpool, \
             tc.tile_pool(name="xs", bufs=4) as xs_pool, \
             tc.tile_pool(name="gates", bufs=2) as gp:

            def step(name, Nb, t):
                grp = "a" if name == "sc" else "b"
                W = w_sb[name]
                h = h_sb[name]
                c_flat = c_sb[name][:].rearrange("p k j -> p (k j)")
                h_flat = h_sb[name][:].rearrange("p k j -> p (k j)")
                x = xs_pool.tile([P, MT, Nb], dt.bfloat16, tag=f"x_{grp}")
                nc.sync.dma_start(x[:], xt[name][t])
                zb = []
                for bank in range(4):
                    z = zpool.tile([P, 4 * Nb], dt.float32, tag=f"z{bank}{grp}")
                    # start=True marks the WHOLE 2KB psum zero-region pending-
                    # zero, so it must appear exactly once per bank per step.
                    for q in range(4):
                        m = 4 * bank + q
                        for k in range(KH):
                            nc.tensor.matmul(
                                z[:, q * Nb:(q + 1) * Nb],
                                lhsT=W[:, k, m * P:(m + 1) * P],
                                rhs=h[:, k, :],
                                start=(q == 0 and k == 0), stop=False,
                                skip_group_check=True)
                    for q in range(4):
                        nc.tensor.matmul(
                            z[:, q * Nb:(q + 1) * Nb],
                            lhsT=ident[:],
                            rhs=x[:, 4 * bank + q, :],
                            start=False, stop=(q == 3),
                            skip_group_check=True)
                    zb.append(z)
                ig = gp.tile([P, 4 * Nb], dt.bfloat16, tag=f"ig_{grp}")
                fg = gp.tile([P, 4 * Nb], dt.bfloat16, tag=f"fg_{grp}")
                og = gp.tile([P, 4 * Nb], dt.bfloat16, tag=f"og_{grp}")
                gg = gp.tile([P, 4 * Nb], dt.bfloat16, tag=f"gg_{grp}")
                gf32 = gp.tile([P, 4 * Nb], dt.float32, tag=f"g32_{grp}")
                nc.scalar.activation(ig[:], zb[0][:], A.Sigmoid)
                nc.scalar.activation(fg[:], zb[1][:], A.Sigmoid)
                nc.scalar.activation(og[:], zb[2][:], A.Sigmoid)
                # tanh(x) = 2*sigmoid(2x) - 1  (keeps ACT on one LUT set).
                # sigma stays fp32 until the affine: bf16 near 0.5 would lose
                # ~all of tanh's significand (quantum 2^-9 vs |tanh| ~ 0.03).
                nc.scalar.activation(gf32[:], zb[3][:], A.Sigmoid, scale=2.0)
                nc.vector.tensor_scalar(gg[:], gf32[:], 2.0, -1.0, OP.mult, OP.add)
                tmp = gp.tile([P, 4 * Nb], dt.bfloat16, tag=f"tm_{grp}")
                nc.vector.tensor_mul(tmp[:], ig[:], gg[:])
                nc.vector.tensor_mul(c_flat, fg[:], c_flat)
                nc.vector.tensor_add(c_flat, c_flat, tmp[:])
                tch = gp.tile([P, 4 * Nb], dt.bfloat16, tag=f"tc_{grp}")
                nc.scalar.activation(gf32[:], c_flat, A.Sigmoid, scale=2.0)
                nc.vector.tensor_scalar(tch[:], gf32[:], 2.0, -1.0, OP.mult, OP.add)
                nc.vector.tensor_mul(h_flat, og[:], tch[:])
                if _DEBUG == 2 and name == "sc" and t == int(os.environ.get("BASSK_DUMPT", "0")):
                    nc.sync.dma_start(dram["dbg0_x"][:], x[:].rearrange("p m j -> p (m j)"))
                    for nm, tile_ in (("ig", ig), ("fg", fg), ("og", og), ("gg", gg)):
                        nc.sync.dma_start(dram[f"dbg0_{nm}"][:], tile_[:])
                    nc.sync.dma_start(dram["dbg0_c"][:], c_flat)
                    nc.sync.dma_start(dram["dbg0_h"][:], h_flat)
                    for bank in range(4):
                        zsb = gp.tile([P, 4 * Nb], dt.float32, tag="zdbg")
                        nc.vector.tensor_copy(zsb[:], zb[bank][:])
                        nc.sync.dma_start(dram[f"dbg0_z{bank}"][:], zsb[:])

            t_sc, t_cm, t_is = (c[1] for c in CHAINS)
            if _PHASE < 3:
                t_sc = t_cm = t_is = 1
            is_done = 0
            for r in range(t_sc):
                step("sc", NSEQ, r)
                if r < t_cm:
                    step("cm", NSEQ, r)
                elif is_done < t_is and (r - t_cm) % 2 == 0:
                    step("is", BPC, is_done)
                    is_done += 1
            for r in range(is_done, t_is):
                step("is", BPC, r)

        if _DEBUG:
            for name, T, Nb, S in CHAINS:
                nc.sync.dma_start(dram[f"dbg_h_{name}"][:], h_sb[name][:])
                nc.sync.dma_start(dram[f"dbg_c_{name}"][:], c_sb[name][:])

        # ---- phase C: merge + final projection ----
        with tc.tile_pool(name="fin", bufs=1) as fin, \
             tc.tile_pool(name="fpsum", bufs=2, space="PSUM") as fp:
            for side, st1, st2, st_is in (
                    (0, h_sb["sc"], h_sb["cm"], h_sb["is"]),
                    (1, c_sb["sc"], c_sb["cm"], c_sb["is"])):
                # hm[j] = hcat[j] . wm  over both halves
                mm = fp.tile([1, NSEQ], dt.float32, tag="mg")
                for half, st in ((0, st1), (1, st2)):
                    for k in range(KH):
                        col = 2 * side + half
                        nc.tensor.matmul(
                            mm[:], lhsT=wm_sb[:, k, col:col + 1], rhs=st[:, k, :],
                            start=(half == 0 and k == 0),
                            stop=(half == 1 and k == KH - 1),
                            skip_group_check=True)
                hm_bf = fin.tile([1, NSEQ], dt.bfloat16, tag=f"hm{side}")
                nc.vector.tensor_scalar(
                    hm_bf[:], mm[:], bm_sb[0:1, side:side + 1], None, OP.add)
                # reshape [80] -> [10, 8] via DRAM bounce; zero-pad to 128 rows
                nc.sync.dma_start(scratch[side][None, :], hm_bf[0:1, :])
                hmT = fin.tile([P, BPC], dt.bfloat16, tag=f"hmT{side}")
                nc.vector.memset(hmT[:], 0.0)
                nc.sync.dma_start(
                    hmT[:NCOM, :], scratch[side].rearrange("(p n) -> n p", n=NCOM))
                out_sb = fin.tile([P, KH, BPC], dt.float32, tag=f"out{side}")
                for m in range(KH):
                    pf = fp.tile([P, BPC], dt.float32, tag="fin")
                    nc.tensor.matmul(
                        pf[:], lhsT=wfm_sb[:, side, m * P:(m + 1) * P], rhs=hmT[:],
                        start=True, stop=False, skip_group_check=True)
                    for k in range(KH):
                        nc.tensor.matmul(
                            pf[:], lhsT=wfh_sb[:, side, k, m * P:(m + 1) * P],
                            rhs=st_is[:, k, :],
                            start=False, stop=(k == KH - 1),
                            skip_group_check=True)
                    nc.scalar.activation(
                        out_sb[:, m, :], pf[:], A.Identity,
                        bias=bf_sb[:, m, side:side + 1])
                nc.sync.dma_start(dram["ho" if side == 0 else "co"][:], out_sb[:])


def _build():
    import concourse.mybir as mybir
    import concourse.tile as tile
    from concourse import bacc

    dt = mybir.dt
    nc = bacc.Bacc("TRN2", target_bir_lowering=False, debug=False,
                   num_devices=NCORES)
    dram = {}
    for name, T, Nb, S in CHAINS:
        dram[f"tab_{name}"] = nc.dram_tensor(f"tab_{name}", [V, E], dt.bfloat16, kind="ExternalInput").ap()
        dram[f"whh_{name}"] = nc.dram_tensor(f"whh_{name}", [H, G], dt.bfloat16, kind="ExternalInput").ap()
        dram[f"wih_{name}"] = nc.dram_tensor(f"wih_{name}", [E, G], dt.bfloat16, kind="ExternalInput").ap()
        dram[f"bias_{name}"] = nc.dram_tensor(f"bias_{name}", [G], dt.float32, kind="ExternalInput").ap()
        dram[f"idx_{name}"] = nc.dram_tensor(f"idx_{name}", [P, T * Nb // 16], dt.int16, kind="ExternalInput").ap()
    dram["wm"] = nc.dram_tensor("wm", [H, 4], dt.bfloat16, kind="ExternalInput").ap()
    dram["bm"] = nc.dram_tensor("bm", [1, 2], dt.float32, kind="ExternalInput").ap()
    dram["wf_m"] = nc.dram_tensor("wf_m", [2, P, H], dt.bfloat16, kind="ExternalInput").ap()
    dram["wf_h"] = nc.dram_tensor("wf_h", [2, H, H], dt.bfloat16, kind="ExternalInput").ap()
    dram["bf"] = nc.dram_tensor("bf", [H, 2], dt.float32, kind="ExternalInput").ap()
    dram["ho"] = nc.dram_tensor("ho", [P, KH, BPC], dt.float32, kind="ExternalOutput").ap()
    dram["co"] = nc.dram_tensor("co", [P, KH, BPC], dt.float32, kind="ExternalOutput").ap()
    if _DEBUG:
        for name, T, Nb, S in CHAINS:
            dram[f"dbg_h_{name}"] = nc.dram_tensor(f"dbg_h_{name}", [P, KH, Nb], dt.bfloat16, kind="ExternalOutput").ap()
            dram[f"dbg_c_{name}"] = nc.dram_tensor(f"dbg_c_{name}", [P, KH, Nb], dt.bfloat16, kind="ExternalOutput").ap()
    if _DEBUG == 2:
        for nm in ("x", "ig", "fg", "og", "gg", "c", "h"):
            dram[f"dbg0_{nm}"] = nc.dram_tensor(f"dbg0_{nm}", [P, MT * NSEQ if nm == "x" else 4 * NSEQ], dt.bfloat16, kind="ExternalOutput").ap()
        for bank in range(4):
            dram[f"dbg0_z{bank}"] = nc.dram_tensor(f"dbg0_z{bank}", [P, 4 * NSEQ], dt.float32, kind="ExternalOutput").ap()

    xt = {}
    for name, T, Nb, S in CHAINS:
        xt[name] = nc.dram_tensor(f"xt_{name}", [T, P, MT, Nb], dt.bfloat16, kind="Internal").ap()
    scratch = [nc.dram_tensor(f"hmsc{i}", [NSEQ], dt.bfloat16, kind="Internal").ap() for i in range(2)]

    with tile.TileContext(nc) as tc:
        _emit(tc, dram, xt, scratch)
    nc.compile()
    return nc


def _prep_inputs(inputs):
    """Build the 8 per-core input maps from full-size inputs."""
    comments = np.asarray(inputs["comments"]).astype(np.int32)
    cm = np.asarray(inputs["cm"]).astype(np.int32)
    issue = np.asarray(inputs["issue"]).astype(np.int32)

    def bf(x):
        return np.ascontiguousarray(np.asarray(x).astype(BF16))

    shared = {}
    for name, src in (("sc", "emb_sc"), ("cm", "emb_cm"), ("is", "emb_is")):
        shared[f"tab_{name}"] = bf(inputs[src])
    for name, whh, wih, b in (("sc", "Whh_sc", "Wih_sc", "b_sc"),
                              ("cm", "Whh_cm", "Wih_cm", "b_cm"),
                              ("is", "Whh_is", "Wih_is", "b_is")):
        Wp = np.asarray(inputs[whh])[_GPERM]            # [G, H] permuted rows
        shared[f"whh_{name}"] = bf(Wp.T)                # [H, G]
        Up = np.asarray(inputs[wih])[_GPERM]
        shared[f"wih_{name}"] = bf(Up.T)                # [E, G]
        shared[f"bias_{name}"] = np.ascontiguousarray(
            np.asarray(inputs[b])[_GPERM].astype(np.float32))
    wm = np.stack([np.asarray(inputs["Wmh"])[0, :H],
                   np.asarray(inputs["Wmh"])[0, H:],
                   np.asarray(inputs["Wmc"])[0, :H],
                   np.asarray(inputs["Wmc"])[0, H:]], axis=1)   # [H, 4]
    shared["wm"] = bf(wm)
    shared["bm"] = np.array([[inputs["bmh"][0], inputs["bmc"][0]]], dtype=np.float32)
    wf_m = np.zeros((2, P, H), np.float32)
    wf_h = np.zeros((2, H, H), np.float32)
    for i, w in enumerate(("Wfh", "Wfc")):
        WT = np.asarray(inputs[w]).T                    # [522, 512]
        wf_m[i, :NCOM] = WT[:NCOM]
        wf_h[i] = WT[NCOM:]
    shared["wf_m"] = bf(wf_m)
    shared["wf_h"] = bf(wf_h)
    shared["bf"] = np.ascontiguousarray(
        np.stack([inputs["bfh"], inputs["bfc"]], axis=1).astype(np.float32))

    def wrap16(flat):
        # dma_gather index layout: idx i -> [i % 16, i // 16], int16,
        # replicated over all 128 partitions (8 gpsimd channels x 16).
        w = flat.reshape(-1, 16).T.astype(np.int16)     # [16, n/16]
        return np.ascontiguousarray(np.tile(w, (P // 16, 1)))

    in_maps = []
    for c in range(NCORES):
        m = dict(shared)
        prs = slice(c * BPC, (c + 1) * BPC)
        # time-major token ids: token f = t*Nb + j, j = pr_local*NCOM + ncom
        sc = comments[prs].reshape(NSEQ, LSC)[:, :CHAINS[0][1]]   # [80, T]
        m["idx_sc"] = wrap16(sc.T.reshape(-1))
        cmv = cm[prs].reshape(NSEQ, LCM)[:, :CHAINS[1][1]]
        m["idx_cm"] = wrap16(cmv.T.reshape(-1))
        isv = issue[prs][:, :CHAINS[2][1]]              # [8, T]
        m["idx_is"] = wrap16(isv.T.reshape(-1))
        in_maps.append(m)
    return in_maps


def kernel(**inputs):
    from concourse.bass_utils import run_bass_kernel_spmd

    in_maps = _prep_inputs(inputs)
    if "nc" not in _CACHE:
        _CACHE["nc"] = _build()
    res = run_bass_kernel_spmd(_CACHE["nc"], in_maps, core_ids=list(range(NCORES)))
    h = np.zeros((B, H), np.float32)
    c = np.zeros((B, H), np.float32)
    for ci, r in enumerate(res.results):
        # ho [128, 4, 8]: ho[p, k, j] = h[8*ci + j, 128*k + p]
        h[ci * BPC:(ci + 1) * BPC] = r["ho"].transpose(2, 1, 0).reshape(BPC, H)
        c[ci * BPC:(ci + 1) * BPC] = r["co"].transpose(2, 1, 0).reshape(BPC, H)
    return h[None], c[None]

